# revision 9
# baseline (speedup 1.0000x reference)
"""BitLinear baseline (layernorm -> sign(W - mean(W)) GEMM -> *beta) on 8 TRN2 cores.

Sharding: data-parallel over tokens. Each core gets 1024 of the 8192 tokens
(x pre-transposed on host to [D_in, T_loc] bf16) and the full W^T in bf16,
both ROTATED along the CONTRACTION dim (d_in) by c*512 rows per core. Since a
contraction is order-invariant, no un-rotation of the output is needed; the
rotation only makes rows 0..511 (i-tiles 0..3) core c's private 1/8 stats
shard of W, loaded once as four contiguous [128, 4096] row tiles that are
retained in SBUF and re-sliced by the main GEMM.

Device-side math (per core):
  One AllReduce of [sum(W), sum(relu(W)), count(W>=0)] over the row shards:
    mu    = sum/N
    sum|w|= 2*relu_sum - sum
    beta  = (sum|w| - mu*(2*count - N))/N   (|w-mu| identity, error O(mu^2))
  The stats run on DVE (3 accumulating tensor_scalar passes over bf16 rows)
  so the AllReduce triggers ~16us in; x load + token-stat matmuls on the
  tensor engine fill the AllReduce latency window.
  out[s,o] = a[s]*(raw[s,o] + mx[s]*colsum[o]),  raw = x @ sign(W-mu)^T
  with a[s] = beta/sqrt(var[s]+eps), mx[s] = -mean_x[s]. The colsum
  correction is folded into each PSUM accumulation group as a rank-1 K=1
  matmul (lhsT = mx row slice, rhs = colsum row), so the epilogue is a
  single DVE tensor_scalar multiply by a[s] straight out of PSUM, written
  as bf16 (host upcasts). a[s] is moved to partition layout with rank-1
  transpose matmuls instead of a DRAM round-trip.
  Matmuls run in bf16 (sign values exact in bf16), accumulation in fp32 PSUM.
"""

import numpy as np
from contextlib import ExitStack

from concourse import bass, bacc, tile, mybir
from concourse.bass_utils import run_bass_kernel_spmd

F32 = mybir.dt.float32
BF16 = mybir.dt.bfloat16
P = 128
LN_EPS = 1e-5

B, S, D_IN, D_OUT = 4, 2048, 4096, 4096
N_CORES = 8
T_TOTAL = B * S
T_LOC = T_TOTAL // N_CORES


def build_program(n_cores, t_loc, d_in, d_out, oc_width=512):
    n_it = d_in // P            # i tiles (contraction)
    n_st = t_loc // P           # s tiles (tokens)
    n_oc = d_out // oc_width    # output-feature chunks
    n_wr = (d_in // n_cores) // P   # row tiles in the per-core stats shard
    inv_w = 1.0 / float(d_in * d_out)
    inv_d = 1.0 / float(d_in)
    groups = [list(range(n_cores))]
    AX = mybir.AxisListType.X
    ADD = mybir.AluOpType.add
    AF = mybir.ActivationFunctionType

    nc = bacc.Bacc("TRN2", target_bir_lowering=False, debug=False,
                   num_devices=n_cores)
    xt = nc.dram_tensor("xt", [d_in, t_loc], BF16, kind="ExternalInput").ap()
    wt = nc.dram_tensor("wt", [d_in, d_out], BF16, kind="ExternalInput").ap()
    out = nc.dram_tensor("out", [t_loc, d_out], BF16,
                         kind="ExternalOutput").ap()

    with tile.TileContext(nc) as tc, ExitStack() as ctx:
        const = ctx.enter_context(tc.tile_pool(name="const", bufs=1))
        persist = ctx.enter_context(tc.tile_pool(name="persist", bufs=1))
        dram = ctx.enter_context(tc.tile_pool(name="dram", bufs=1, space="DRAM"))

        ones_col_bf = const.tile([P, 1], BF16, tag="ones_col_bf")
        nc.vector.memset(ones_col_bf[:], 1.0)
        ones_col_f = const.tile([P, 1], F32, tag="ones_col_f")
        nc.vector.memset(ones_col_f[:], 1.0)
        ones_row_f = const.tile([1, P], F32, tag="ones_row_f")
        nc.vector.memset(ones_row_f[:], 1.0)
        one_f = const.tile([1, 1], F32, tag="one_f")
        nc.vector.memset(one_f[:], 1.0)
        eps_col = const.tile([P, 1], F32, tag="eps_col")
        nc.vector.memset(eps_col[:], LN_EPS)

        neg_mu = persist.tile([P, 1], F32, tag="neg_mu")
        beta_col = persist.tile([P, 1], F32, tag="beta_col")
        a_col = persist.tile([P, n_st], F32, tag="a_col")
        mx_row = persist.tile([1, t_loc], F32, tag="mx_row")
        cs_row_p = ctx.enter_context(tc.tile_pool(name="csrow", bufs=2))

        xbf_pool = ctx.enter_context(tc.tile_pool(name="xbf", bufs=1))
        wrow_pool = ctx.enter_context(tc.tile_pool(name="wrow", bufs=1))
        wload = ctx.enter_context(tc.tile_pool(name="wload", bufs=12))
        wbin_pool = ctx.enter_context(
            tc.tile_pool(name="wbin", bufs=2 * n_it))
        tree_pool = ctx.enter_context(tc.tile_pool(name="tree", bufs=2))
        outsb = ctx.enter_context(tc.tile_pool(name="outsb", bufs=4))

        # ---- Phase 1: W stats from the contiguous row shard (i-tiles 0..3) --
        wrows = []
        ps12_ctx = ExitStack()
        ps12 = ps12_ctx.enter_context(
            tc.tile_pool(name="ps12", bufs=1, space="PSUM"))
        ps3_ctx = ExitStack()
        ps3 = ps3_ctx.enter_context(
            tc.tile_pool(name="ps3", bufs=1, space="PSUM"))
        with tc.tile_pool(name="wstat", bufs=1) as wstat, \
             tc.tile_pool(name="wscr", bufs=1) as wscr:
            racc = wstat.tile([P, n_wr], F32, tag="racc")
            iacc = wstat.tile([P, n_wr], F32, tag="iacc")
            sacc = wstat.tile([P, n_wr], F32, tag="sacc")
            for i in range(n_wr):
                wr = wrow_pool.tile([P, d_out], BF16, tag=f"wrow{i}")
                nc.sync.dma_start(wr[:], wt[i * P:(i + 1) * P, :])
                wrows.append(wr)
                scr = wscr.tile([P, d_out], BF16, tag="scr")
                nc.vector.tensor_scalar(scr[:], wr[:], 0.0, 0.0,
                                        mybir.AluOpType.max, ADD,
                                        accum_out=racc[:, i:i + 1])
                scr2 = wscr.tile([P, d_out], BF16, tag="scr")
                nc.vector.tensor_scalar(scr2[:], wr[:], 0.0, 0.0,
                                        mybir.AluOpType.is_ge, ADD,
                                        accum_out=iacc[:, i:i + 1])
                scr3 = wscr.tile([P, d_out], BF16, tag="scr")
                nc.vector.tensor_scalar(scr3[:], wr[:], 0.0, 0.0,
                                        ADD, ADD,
                                        accum_out=sacc[:, i:i + 1])
            s3 = wstat.tile([P, 3], F32, tag="s3")
            nc.vector.tensor_reduce(s3[:, 0:1], sacc[:], axis=AX, op=ADD)
            nc.vector.tensor_reduce(s3[:, 1:2], racc[:], axis=AX, op=ADD)
            nc.vector.tensor_reduce(s3[:, 2:3], iacc[:], axis=AX, op=ADD)
            ps_tot = ps12.tile([1, 3], F32, tag="ps_tot")
            nc.tensor.matmul(ps_tot[:], ones_col_f[:], s3[:])
            sb_tot = wstat.tile([1, 3], F32, tag="sb_tot")
            nc.vector.tensor_copy(sb_tot[:], ps_tot[:])
            ar_in = dram.tile([1, 3], F32, tag="ar_in")
            ar_out = dram.tile([1, 3], F32, tag="ar_out")
            nc.scalar.dma_start(ar_in[:], sb_tot[:])
            nc.gpsimd.collective_compute(
                "AllReduce", ADD, replica_groups=groups,
                ins=[ar_in.opt()], outs=[ar_out.opt()])

        # ---- Phase 2: x load (bf16) + token stats on PE (fills AR window) --
        xbf_tiles = []
        n_ch = t_loc // 512
        statsb_ctx = ExitStack()
        statsb = statsb_ctx.enter_context(tc.tile_pool(name="statsb", bufs=1))
        with tc.tile_pool(name="x2p", bufs=2) as x2p:
            ps_s = ps3.tile([1, t_loc], F32, tag="ps_s")
            ps_s2 = ps3.tile([1, t_loc], F32, tag="ps_s2")
            for i in range(n_it):
                xb = xbf_pool.tile([P, t_loc], BF16, tag=f"xb{i}")
                nc.sync.dma_start(xb[:], xt[i * P:(i + 1) * P, :])
                xbf_tiles.append(xb)
                x2 = x2p.tile([P, t_loc], BF16, tag="x2")
                if i % 2 == 0:
                    nc.scalar.square(x2[:], xb[:])
                else:
                    nc.vector.tensor_mul(x2[:], xb[:], xb[:])
                for c in range(n_ch):
                    sl = slice(c * 512, (c + 1) * 512)
                    nc.tensor.matmul(ps_s[:, sl], ones_col_bf[:], xb[:, sl],
                                     start=(i == 0), stop=(i == n_it - 1))
                    nc.tensor.matmul(ps_s2[:, sl], ones_col_bf[:], x2[:, sl],
                                     start=(i == 0), stop=(i == n_it - 1))

            # ---- token-stat rows -> SBUF; mx_row in row space --------------
            srow = statsb.tile([1, t_loc], F32, tag="srow")
            nc.vector.tensor_copy(srow[:], ps_s[:])
            s2row = statsb.tile([1, t_loc], F32, tag="s2row")
            nc.vector.tensor_copy(s2row[:], ps_s2[:])
            # mx = -mean_x (row layout, feeds the rank-1 colsum correction)
            nc.scalar.activation(mx_row[:], srow[:], AF.Copy, scale=-inv_d)

            # ---- transpose token sums to partition layout via rank-1 MMs ---
            ps_tc = ps12.tile([P, 2 * n_st], F32, tag="ps_tc")
            for s in range(n_st):
                nc.tensor.matmul(ps_tc[:, s:s + 1],
                                 srow[0:1, s * P:(s + 1) * P], one_f[:])
                nc.tensor.matmul(ps_tc[:, n_st + s:n_st + s + 1],
                                 s2row[0:1, s * P:(s + 1) * P], one_f[:])

            # ---- Post-AR scalars (beta, -mu) -------------------------------
            tot = statsb.tile([1, 3], F32, tag="tot")
            nc.scalar.dma_start(tot[:], ar_out[:])
            ps_b = ps12.tile([P, 1], F32, tag="ps_b")
            nc.tensor.matmul(ps_b[:], ones_row_f[:], tot[:, 0:1])
            nc.scalar.mul(neg_mu[:], ps_b[:], -inv_w)
            mu_sb = statsb.tile([1, 1], F32, tag="mu_sb")
            nc.scalar.mul(mu_sb[:], tot[:, 0:1], inv_w)
            # sum|w| = 2*relu_sum - sum
            r2 = statsb.tile([1, 1], F32, tag="r2")
            nc.scalar.mul(r2[:], tot[:, 1:2], 2.0)
            asum = statsb.tile([1, 1], F32, tag="asum")
            nc.vector.tensor_sub(asum[:], r2[:], tot[:, 0:1])
            sgn_t = statsb.tile([1, 1], F32, tag="sgn_t")
            nc.scalar.activation(sgn_t[:], tot[:, 2:3], AF.Copy,
                                 scale=2.0, bias=-float(d_in * d_out))
            t1 = statsb.tile([1, 1], F32, tag="t1")
            nc.vector.tensor_mul(t1[:], mu_sb[:], sgn_t[:])
            t2 = statsb.tile([1, 1], F32, tag="t2")
            nc.vector.tensor_sub(t2[:], asum[:], t1[:])
            beta_sb = statsb.tile([1, 1], F32, tag="beta_sb")
            nc.scalar.mul(beta_sb[:], t2[:], inv_w)
            ps_bc = ps12.tile([P, 1], F32, tag="ps_bc")
            nc.tensor.matmul(ps_bc[:], ones_row_f[:], beta_sb[:])
            nc.vector.tensor_copy(beta_col[:], ps_bc[:])

            # ---- a_col in partition layout ---------------------------------
            mu_c = statsb.tile([P, n_st], F32, tag="mu_c")
            nc.vector.tensor_scalar_mul(mu_c[:], ps_tc[:, 0:n_st], inv_d)
            e2_c = statsb.tile([P, n_st], F32, tag="e2_c")
            nc.vector.tensor_scalar_mul(e2_c[:], ps_tc[:, n_st:2 * n_st],
                                        inv_d)
            m2_c = statsb.tile([P, n_st], F32, tag="m2_c")
            nc.vector.tensor_mul(m2_c[:], mu_c[:], mu_c[:])
            var_c = statsb.tile([P, n_st], F32, tag="var_c")
            nc.vector.tensor_sub(var_c[:], e2_c[:], m2_c[:])
            sd_c = statsb.tile([P, n_st], F32, tag="sd_c")
            nc.scalar.activation(sd_c[:], var_c[:], AF.Sqrt, bias=eps_col[:])
            rs_c = statsb.tile([P, n_st], F32, tag="rs_c")
            nc.vector.reciprocal(rs_c[:], sd_c[:])
            nc.vector.tensor_scalar_mul(a_col[:], rs_c[:], beta_col[:])

        # ---- Phase 3: main GEMM over o-chunks ------------------------------
        statsb_ctx.close()
        ps3_ctx.close()
        ps12_ctx.close()
        ps_main = ctx.enter_context(
            tc.tile_pool(name="ps4", bufs=6, space="PSUM"))
        ps_csp = ctx.enter_context(
            tc.tile_pool(name="ps4c", bufs=1, space="PSUM"))

        def emit_signs(oc):
            o0 = oc * oc_width
            wb = [wbin_pool.tile([P, oc_width], BF16, tag="wb", name="wb")
                  for _ in range(n_it)]
            for i in range(n_it):
                if i < n_wr:
                    src = wrows[i][:, o0:o0 + oc_width]
                else:
                    wf = wload.tile([P, oc_width], BF16, tag="wf")
                    nc.sync.dma_start(
                        wf[:], wt[i * P:(i + 1) * P, o0:o0 + oc_width])
                    src = wf[:]
                nc.scalar.activation(wb[i][:], src, AF.Sign, bias=neg_mu[:])
            return wb

        def emit_colsum(wb):
            # colsum: grouped DVE adds (small ints, exact in bf16)
            ngrp = min(4, n_it)
            per = n_it // ngrp
            gacc = tree_pool.tile([P, ngrp, oc_width], BF16, tag="gacc")
            for g in range(ngrp):
                base = g * per
                nc.vector.tensor_add(gacc[:, g, :], wb[base][:],
                                     wb[base + 1][:])
                for k in range(2, per):
                    nc.vector.tensor_add(gacc[:, g, :], gacc[:, g, :],
                                         wb[base + k][:])
            for g in range(1, ngrp):
                nc.vector.tensor_add(gacc[:, 0, :], gacc[:, 0, :],
                                     gacc[:, g, :])
            cs_ps = ps_csp.tile([1, oc_width], F32, tag="cs_ps")
            nc.tensor.matmul(cs_ps[:], ones_col_bf[:], gacc[:, 0, :])
            cs_row = cs_row_p.tile([1, oc_width], F32, tag="cs_row")
            nc.vector.tensor_copy(cs_row[:], cs_ps[:])
            return cs_row

        def emit_corr(po, cs_row, s):
            # rank-1 colsum correction: po += mx[s-tile] (x) cs_row
            nc.tensor.matmul(po[:], mx_row[0:1, s * P:(s + 1) * P],
                             cs_row[:], start=False, stop=True)

        def emit_epilogue(po, s, o0):
            ob = outsb.tile([P, oc_width], BF16, tag="ob")
            nc.vector.tensor_scalar_mul(ob[:], po[:], a_col[:, s:s + 1])
            nc.gpsimd.dma_start(out[s * P:(s + 1) * P, o0:o0 + oc_width],
                                ob[:])

        for oc in range(n_oc):
            o0 = oc * oc_width
            wb = emit_signs(oc)
            if oc == 0:
                # i-outer: consume sign tiles as they stream out of ACT;
                # colsum emitted AFTER the first group's accumulation so it
                # can't block the PE queue while the DVE tree waits on all
                # signs.
                grp = min(4, n_st)
                cs_row = None
                for h in range(0, n_st, grp):
                    pos = [ps_main.tile([P, oc_width], F32, tag="po",
                                        name="po") for _ in range(grp)]
                    for i in range(n_it):
                        for g in range(grp):
                            s = h + g
                            nc.tensor.matmul(
                                pos[g][:], xbf_tiles[i][:, s * P:(s + 1) * P],
                                wb[i][:], start=(i == 0), stop=False)
                    if h == 0:
                        cs_row = emit_colsum(wb)
                    for g in range(grp):
                        emit_corr(pos[g], cs_row, h + g)
                        emit_epilogue(pos[g], h + g, o0)
            else:
                cs_row = emit_colsum(wb)
                for s in range(n_st):
                    po = ps_main.tile([P, oc_width], F32, tag="po")
                    for i in range(n_it):
                        nc.tensor.matmul(po[:],
                                         xbf_tiles[i][:, s * P:(s + 1) * P],
                                         wb[i][:],
                                         start=(i == 0), stop=False)
                    emit_corr(po, cs_row, s)
                    emit_epilogue(po, s, o0)

    nc.compile()
    return nc


_PROGRAM_CACHE = {}


def _get_program(key):
    if key not in _PROGRAM_CACHE:
        _PROGRAM_CACHE[key] = build_program(*key)
    return _PROGRAM_CACHE[key]


def make_in_maps(x2d, weight, n_cores, t_loc):
    """Host-side sharding: token shards of x^T in bf16; full W^T in bf16.
    Both are rotated along d_in by c*(d_in/n_cores) rows so the program's
    row tiles 0..3 are core c's private 1/8 stats shard. Rotating the
    contraction dim leaves the output invariant, so no un-rotation."""
    bf16 = mybir.dt.np(BF16)
    d_in = x2d.shape[1]
    roll = d_in // n_cores
    wt_full = np.ascontiguousarray(weight.T).astype(bf16)
    in_maps = []
    for c in range(n_cores):
        xt_c = np.ascontiguousarray(
            np.roll(x2d[c * t_loc:(c + 1) * t_loc, :].T, -c * roll, axis=0)
        ).astype(bf16)
        wt_c = np.ascontiguousarray(np.roll(wt_full, -c * roll, axis=0))
        in_maps.append({"xt": xt_c, "wt": wt_c})
    return in_maps


def kernel(x: np.ndarray, weight: np.ndarray) -> np.ndarray:
    assert x.shape == (B, S, D_IN) and weight.shape == (D_OUT, D_IN)
    nc = _get_program((N_CORES, T_LOC, D_IN, D_OUT))
    x2d = np.ascontiguousarray(x.reshape(T_TOTAL, D_IN), dtype=np.float32)
    in_maps = make_in_maps(x2d, weight, N_CORES, T_LOC)
    try:
        res = run_bass_kernel_spmd(nc, in_maps, list(range(N_CORES)),
                                   trace=False)
    except Exception:
        res = run_bass_kernel_spmd(nc, in_maps, list(range(N_CORES)),
                                   trace=False)
    out = np.concatenate(
        [res.results[c]["out"].astype(np.float32) for c in range(N_CORES)],
        axis=0)
    return np.ascontiguousarray(out.reshape(B, S, D_OUT))


# revision 16
# speedup vs baseline: 1.0218x; 1.0218x over previous
"""BitLinear baseline (layernorm -> sign(W - mean(W)) GEMM -> *beta) on 8 TRN2 cores.

Sharding: data-parallel over tokens. Each core gets 1024 of the 8192 tokens
(x pre-transposed on host to [D_in, T_loc] bf16) and the full W^T in bf16,
both ROTATED along the CONTRACTION dim (d_in) by c*512 rows per core. Since a
contraction is order-invariant, no un-rotation of the output is needed; the
rotation only makes rows 0..511 (i-tiles 0..3) core c's private 1/8 stats
shard of W, loaded once as four contiguous [128, 4096] row tiles that are
retained in SBUF and re-sliced by the main GEMM.

Device-side math (per core):
  One AllReduce of [sum(W), sum|W|, count(W>=0)] over the row shards:
    mu    = sum/N
    beta  = (sum|w| - mu*(2*count - N))/N   (|w-mu| identity, error O(mu^2))
  Stats are split across engines so the AllReduce triggers ~20us in:
  sum on PE (ones-matmuls folded into one PSUM bank), |w| on ACT with
  accum_out, count on DVE is_ge with accum_out. x load (4 big DMAs) +
  token-stat matmuls fill the AllReduce latency window.
  out[s,o] = a[s]*(raw[s,o] + mx[s]*colsum[o]),  raw = x @ sign(W-mu)^T
  with a[s] = beta/sqrt(var[s]+eps), mx[s] = -mean_x[s]. The colsum
  correction is folded into each PSUM accumulation group as a rank-1 K=1
  matmul (lhsT = mx row slice, rhs = colsum row), so the epilogue is a
  single DVE tensor_scalar multiply by a[s] straight out of PSUM, written
  as bf16 (host upcasts). a[s] is moved to partition layout with rank-1
  transpose matmuls instead of a DRAM round-trip.
  Matmuls run in bf16 (sign values exact in bf16), accumulation in fp32 PSUM.
"""

import numpy as np
from contextlib import ExitStack

from concourse import bass, bacc, tile, mybir
from concourse.bass_utils import run_bass_kernel_spmd

F32 = mybir.dt.float32
BF16 = mybir.dt.bfloat16
P = 128
LN_EPS = 1e-5

B, S, D_IN, D_OUT = 4, 2048, 4096, 4096
N_CORES = 8
T_TOTAL = B * S
T_LOC = T_TOTAL // N_CORES


def build_program(n_cores, t_loc, d_in, d_out, oc_width=512):
    n_it = d_in // P            # i tiles (contraction)
    n_st = t_loc // P           # s tiles (tokens)
    n_oc = d_out // oc_width    # output-feature chunks
    n_wr = (d_in // n_cores) // P   # row tiles in the per-core stats shard
    n_xp = 4                    # x load pieces
    it_per_xp = n_it // n_xp
    grp_w = 4                   # i tiles per grouped wf DMA / sign op
    n_wg = n_it // grp_w
    inv_w = 1.0 / float(d_in * d_out)
    inv_d = 1.0 / float(d_in)
    groups = [list(range(n_cores))]
    AX = mybir.AxisListType.X
    ADD = mybir.AluOpType.add
    AF = mybir.ActivationFunctionType

    nc = bacc.Bacc("TRN2", target_bir_lowering=False, debug=False,
                   num_devices=n_cores)
    xt = nc.dram_tensor("xt", [d_in, t_loc], BF16, kind="ExternalInput").ap()
    wt = nc.dram_tensor("wt", [d_in, d_out], BF16, kind="ExternalInput").ap()
    out = nc.dram_tensor("out", [t_loc, d_out], BF16,
                         kind="ExternalOutput").ap()

    with tile.TileContext(nc) as tc, ExitStack() as ctx:
        const = ctx.enter_context(tc.tile_pool(name="const", bufs=1))
        persist = ctx.enter_context(tc.tile_pool(name="persist", bufs=1))
        dram = ctx.enter_context(tc.tile_pool(name="dram", bufs=1, space="DRAM"))

        ones_col_bf = const.tile([P, 1], BF16, tag="ones_col_bf")
        nc.vector.memset(ones_col_bf[:], 1.0)
        ones_col_f = const.tile([P, 1], F32, tag="ones_col_f")
        nc.vector.memset(ones_col_f[:], 1.0)
        ones_row_f = const.tile([1, P], F32, tag="ones_row_f")
        nc.vector.memset(ones_row_f[:], 1.0)
        one_f = const.tile([1, 1], F32, tag="one_f")
        nc.vector.memset(one_f[:], 1.0)
        eps_col = const.tile([P, 1], F32, tag="eps_col")
        nc.vector.memset(eps_col[:], LN_EPS)
        zero_c = const.tile([P, 1], F32, tag="zero_c")
        nc.vector.memset(zero_c[:], 0.0)

        neg_mu = persist.tile([P, 1], F32, tag="neg_mu")
        beta_col = persist.tile([P, 1], F32, tag="beta_col")
        a_col = persist.tile([P, n_st], F32, tag="a_col")
        mx_row = persist.tile([1, t_loc], F32, tag="mx_row")
        cs_row_p = ctx.enter_context(tc.tile_pool(name="csrow", bufs=2))

        xbf_pool = ctx.enter_context(tc.tile_pool(name="xbf", bufs=1))
        wrow_pool = ctx.enter_context(tc.tile_pool(name="wrow", bufs=1))
        wload = ctx.enter_context(tc.tile_pool(name="wload", bufs=3))
        wbin_pool = ctx.enter_context(
            tc.tile_pool(name="wbin", bufs=2 * n_wg))
        tree_pool = ctx.enter_context(tc.tile_pool(name="tree", bufs=1))
        outsb = ctx.enter_context(tc.tile_pool(name="outsb", bufs=4))

        # ---- Phase 1: W stats from the contiguous row shard (i-tiles 0..3) --
        # sum(w) on PE (folded mod-512 into one PSUM bank), |w| on ACT with
        # accum_out, count(w>=0) on DVE with accum_out -- all three engines
        # chew the shard in parallel so the AllReduce triggers early.
        wrows = []
        ps12_ctx = ExitStack()
        ps12 = ps12_ctx.enter_context(
            tc.tile_pool(name="ps12", bufs=1, space="PSUM"))
        ps3_ctx = ExitStack()
        ps3 = ps3_ctx.enter_context(
            tc.tile_pool(name="ps3", bufs=1, space="PSUM"))
        with tc.tile_pool(name="wstat", bufs=1) as wstat, \
             tc.tile_pool(name="wscr", bufs=1) as wscr:
            aacc = wstat.tile([P, 2 * n_wr], F32, tag="aacc")
            iacc = wstat.tile([P, n_wr], F32, tag="iacc")
            ps_sum = ps12.tile([1, oc_width], F32, tag="ps_sum")
            ascr = wscr.tile([P, d_out // 2], BF16, tag="ascr")
            iscr = wscr.tile([P, d_out], BF16, tag="iscr")
            for i in range(n_wr):
                wr = wrow_pool.tile([P, d_out], BF16, tag=f"wrow{i}")
                nc.sync.dma_start(wr[:], wt[i * P:(i + 1) * P, :])
                wrows.append(wr)
                for j in range(n_oc):
                    nc.tensor.matmul(
                        ps_sum[:], ones_col_bf[:],
                        wr[:, j * oc_width:(j + 1) * oc_width],
                        start=(i == 0 and j == 0),
                        stop=(i == n_wr - 1 and j == n_oc - 1))
                for h in range(2):
                    nc.scalar.activation(
                        ascr[:], wr[:, h * (d_out // 2):(h + 1) * (d_out // 2)],
                        AF.Abs, bias=zero_c[:],
                        accum_out=aacc[:, 2 * i + h:2 * i + h + 1])
                nc.vector.tensor_scalar(iscr[:], wr[:], 0.0, 0.0,
                                        mybir.AluOpType.is_ge, ADD,
                                        accum_out=iacc[:, i:i + 1])
            s3 = wstat.tile([P, 2], F32, tag="s3")
            nc.vector.tensor_reduce(s3[:, 0:1], aacc[:], axis=AX, op=ADD)
            nc.vector.tensor_reduce(s3[:, 1:2], iacc[:], axis=AX, op=ADD)
            # one shared small-PSUM tile (bank-granular allocator): cols
            # 0..15 = transposed token sums, 16 = -mu bcast, 17 = beta
            # bcast, 18..19 = [sum|w|, count] totals on partition 0
            ps_tc = ps12.tile([P, 2 * n_st + 4], F32, tag="ps_tc")
            ps_tot = ps_tc[0:1, 2 * n_st + 2:2 * n_st + 4]
            nc.tensor.matmul(ps_tot, ones_col_f[:], s3[:])
            sb_tot = wstat.tile([1, 3], F32, tag="sb_tot")
            sscr = wstat.tile([1, oc_width], F32, tag="sscr")
            nc.scalar.activation(sscr[:], ps_sum[:], AF.Copy, bias=0.0,
                                 accum_out=sb_tot[:, 0:1])
            nc.vector.tensor_copy(sb_tot[:, 1:3], ps_tot)
            ar_in = dram.tile([1, 3], F32, tag="ar_in")
            ar_out = dram.tile([1, 3], F32, tag="ar_out")
            nc.scalar.dma_start(ar_in[:], sb_tot[:])
            nc.gpsimd.collective_compute(
                "AllReduce", ADD, replica_groups=groups,
                ins=[ar_in.opt()], outs=[ar_out.opt()])

        # ---- Phase 2: x load (bf16, 4 big DMAs) + token stats on PE --------
        xpieces = []
        n_ch = t_loc // 512
        statsb_ctx = ExitStack()
        statsb = statsb_ctx.enter_context(tc.tile_pool(name="statsb", bufs=1))
        with tc.tile_pool(name="x2p", bufs=2) as x2p:
            ps_s = ps3.tile([1, t_loc], F32, tag="ps_s")
            ps_s2 = ps3.tile([1, t_loc], F32, tag="ps_s2")
            for k in range(n_xp):
                xp = xbf_pool.tile([P, it_per_xp, t_loc], BF16, tag=f"xp{k}")
                nc.sync.dma_start(
                    xp[:],
                    xt[k * it_per_xp * P:(k + 1) * it_per_xp * P, :]
                    .rearrange("(j p) t -> p j t", p=P))
                xpieces.append(xp)
                for j in range(it_per_xp):
                    i = k * it_per_xp + j
                    x2 = x2p.tile([P, t_loc], BF16, tag="x2")
                    if i % 2 == 0:
                        nc.scalar.square(x2[:], xp[:, j, :])
                    else:
                        nc.vector.tensor_mul(x2[:], xp[:, j, :], xp[:, j, :])
                    for c in range(n_ch):
                        sl = slice(c * 512, (c + 1) * 512)
                        nc.tensor.matmul(ps_s[:, sl], ones_col_bf[:],
                                         xp[:, j, sl],
                                         start=(i == 0), stop=(i == n_it - 1))
                        nc.tensor.matmul(ps_s2[:, sl], ones_col_bf[:],
                                         x2[:, sl],
                                         start=(i == 0), stop=(i == n_it - 1))

            def xtile(i):
                return xpieces[i // it_per_xp][:, i % it_per_xp, :]

            # ---- token-stat rows -> SBUF; mx_row in row space --------------
            srow = statsb.tile([1, t_loc], F32, tag="srow")
            nc.vector.tensor_copy(srow[:], ps_s[:])
            s2row = statsb.tile([1, t_loc], F32, tag="s2row")
            nc.vector.tensor_copy(s2row[:], ps_s2[:])
            # mx = -mean_x (row layout, feeds the rank-1 colsum correction)
            nc.scalar.activation(mx_row[:], srow[:], AF.Copy, scale=-inv_d)

            # ---- transpose token sums to partition layout via rank-1 MMs ---
            for s in range(n_st):
                nc.tensor.matmul(ps_tc[:, s:s + 1],
                                 srow[0:1, s * P:(s + 1) * P], one_f[:])
                nc.tensor.matmul(ps_tc[:, n_st + s:n_st + s + 1],
                                 s2row[0:1, s * P:(s + 1) * P], one_f[:])

            # ---- Post-AR scalars (beta, -mu) -------------------------------
            tot = statsb.tile([1, 3], F32, tag="tot")
            nc.scalar.dma_start(tot[:], ar_out[:])
            ps_b = ps_tc[:, 2 * n_st:2 * n_st + 1]
            nc.tensor.matmul(ps_b, ones_row_f[:], tot[:, 0:1])
            nc.scalar.mul(neg_mu[:], ps_b, -inv_w)
            mu_sb = statsb.tile([1, 1], F32, tag="mu_sb")
            nc.scalar.mul(mu_sb[:], tot[:, 0:1], inv_w)
            sgn_t = statsb.tile([1, 1], F32, tag="sgn_t")
            nc.scalar.activation(sgn_t[:], tot[:, 2:3], AF.Copy,
                                 scale=2.0, bias=-float(d_in * d_out))
            t1 = statsb.tile([1, 1], F32, tag="t1")
            nc.vector.tensor_mul(t1[:], mu_sb[:], sgn_t[:])
            t2 = statsb.tile([1, 1], F32, tag="t2")
            nc.vector.tensor_sub(t2[:], tot[:, 1:2], t1[:])
            beta_sb = statsb.tile([1, 1], F32, tag="beta_sb")
            nc.scalar.mul(beta_sb[:], t2[:], inv_w)
            ps_bc = ps_tc[:, 2 * n_st + 1:2 * n_st + 2]
            nc.tensor.matmul(ps_bc, ones_row_f[:], beta_sb[:])
            nc.vector.tensor_copy(beta_col[:], ps_bc)

            # ---- a_col in partition layout ---------------------------------
            mu_c = statsb.tile([P, n_st], F32, tag="mu_c")
            nc.vector.tensor_scalar_mul(mu_c[:], ps_tc[:, 0:n_st], inv_d)
            e2_c = statsb.tile([P, n_st], F32, tag="e2_c")
            nc.vector.tensor_scalar_mul(e2_c[:], ps_tc[:, n_st:2 * n_st],
                                        inv_d)
            m2_c = statsb.tile([P, n_st], F32, tag="m2_c")
            nc.vector.tensor_mul(m2_c[:], mu_c[:], mu_c[:])
            var_c = statsb.tile([P, n_st], F32, tag="var_c")
            nc.vector.tensor_sub(var_c[:], e2_c[:], m2_c[:])
            sd_c = statsb.tile([P, n_st], F32, tag="sd_c")
            nc.scalar.activation(sd_c[:], var_c[:], AF.Sqrt, bias=eps_col[:])
            rs_c = statsb.tile([P, n_st], F32, tag="rs_c")
            nc.vector.reciprocal(rs_c[:], sd_c[:])
            nc.vector.tensor_scalar_mul(a_col[:], rs_c[:], beta_col[:])

        # ---- Phase 3: main GEMM over o-chunks ------------------------------
        statsb_ctx.close()
        ps3_ctx.close()
        ps12_ctx.close()
        ps_main = ctx.enter_context(
            tc.tile_pool(name="ps4", bufs=6, space="PSUM"))
        ps_csp = ctx.enter_context(
            tc.tile_pool(name="ps4c", bufs=1, space="PSUM"))

        def emit_signs(oc):
            # one [128, grp_w*oc_width] sign tile per group of grp_w i-tiles
            o0 = oc * oc_width
            wgs = []
            for g in range(n_wg):
                wg = wbin_pool.tile([P, grp_w, oc_width], BF16, tag="wb",
                                    name="wb")
                if g == 0:
                    for j in range(grp_w):
                        nc.scalar.activation(wg[:, j, :],
                                             wrows[j][:, o0:o0 + oc_width],
                                             AF.Sign, bias=neg_mu[:])
                else:
                    wf = wload.tile([P, grp_w, oc_width], BF16, tag="wf")
                    nc.sync.dma_start(
                        wf[:],
                        wt[g * grp_w * P:(g + 1) * grp_w * P, o0:o0 + oc_width]
                        .rearrange("(j p) o -> p j o", p=P))
                    nc.scalar.activation(wg[:, :, :], wf[:], AF.Sign,
                                         bias=neg_mu[:])
                wgs.append(wg)
            return wgs

        def wbtile(wgs, i):
            return wgs[i // grp_w][:, i % grp_w, :]

        def emit_colsum(wgs):
            # colsum: grouped DVE adds (small ints, exact in bf16)
            gacc = tree_pool.tile([P, grp_w, oc_width], BF16, tag="gacc")
            nc.vector.tensor_add(gacc[:], wgs[0][:], wgs[1][:])
            for g in range(2, n_wg):
                nc.vector.tensor_add(gacc[:], gacc[:], wgs[g][:])
            for j in range(1, grp_w):
                nc.vector.tensor_add(gacc[:, 0, :], gacc[:, 0, :],
                                     gacc[:, j, :])
            cs_ps = ps_csp.tile([1, oc_width], F32, tag="cs_ps")
            nc.tensor.matmul(cs_ps[:], ones_col_bf[:], gacc[:, 0, :])
            cs_row = cs_row_p.tile([1, oc_width], F32, tag="cs_row")
            nc.vector.tensor_copy(cs_row[:], cs_ps[:])
            return cs_row

        def emit_corr(po, cs_row, s):
            # rank-1 colsum correction: po += mx[s-tile] (x) cs_row
            nc.tensor.matmul(po[:], mx_row[0:1, s * P:(s + 1) * P],
                             cs_row[:], start=False, stop=True)

        def emit_epilogue(po, s, o0):
            ob = outsb.tile([P, oc_width], BF16, tag="ob")
            nc.vector.tensor_scalar_mul(ob[:], po[:], a_col[:, s:s + 1])
            nc.gpsimd.dma_start(out[s * P:(s + 1) * P, o0:o0 + oc_width],
                                ob[:])

        for oc in range(n_oc):
            o0 = oc * oc_width
            wgs = emit_signs(oc)
            if oc == 0:
                # i-outer: consume sign tiles as they stream out of ACT;
                # colsum emitted AFTER the first group's accumulation so it
                # can't block the PE queue while the DVE tree waits on all
                # signs.
                grp = min(4, n_st)
                cs_row = None
                for h in range(0, n_st, grp):
                    pos = [ps_main.tile([P, oc_width], F32, tag="po",
                                        name="po") for _ in range(grp)]
                    for i in range(n_it):
                        for g in range(grp):
                            s = h + g
                            nc.tensor.matmul(
                                pos[g][:], xtile(i)[:, s * P:(s + 1) * P],
                                wbtile(wgs, i)[:],
                                start=(i == 0), stop=False)
                    if h == 0:
                        cs_row = emit_colsum(wgs)
                    for g in range(grp):
                        emit_corr(pos[g], cs_row, h + g)
                        emit_epilogue(pos[g], h + g, o0)
            else:
                cs_row = emit_colsum(wgs)
                for s in range(n_st):
                    po = ps_main.tile([P, oc_width], F32, tag="po")
                    for i in range(n_it):
                        nc.tensor.matmul(po[:],
                                         xtile(i)[:, s * P:(s + 1) * P],
                                         wbtile(wgs, i)[:],
                                         start=(i == 0), stop=False)
                    emit_corr(po, cs_row, s)
                    emit_epilogue(po, s, o0)

    nc.compile()
    return nc


_PROGRAM_CACHE = {}


def _get_program(key):
    if key not in _PROGRAM_CACHE:
        _PROGRAM_CACHE[key] = build_program(*key)
    return _PROGRAM_CACHE[key]


def make_in_maps(x2d, weight, n_cores, t_loc):
    """Host-side sharding: token shards of x^T in bf16; full W^T in bf16.
    Both are rotated along d_in by c*(d_in/n_cores) rows so the program's
    row tiles 0..3 are core c's private 1/8 stats shard. Rotating the
    contraction dim leaves the output invariant, so no un-rotation."""
    bf16 = mybir.dt.np(BF16)
    d_in = x2d.shape[1]
    roll = d_in // n_cores
    wt_full = np.ascontiguousarray(weight.T).astype(bf16)
    in_maps = []
    for c in range(n_cores):
        xt_c = np.ascontiguousarray(
            np.roll(x2d[c * t_loc:(c + 1) * t_loc, :].T, -c * roll, axis=0)
        ).astype(bf16)
        wt_c = np.ascontiguousarray(np.roll(wt_full, -c * roll, axis=0))
        in_maps.append({"xt": xt_c, "wt": wt_c})
    return in_maps


def kernel(x: np.ndarray, weight: np.ndarray) -> np.ndarray:
    assert x.shape == (B, S, D_IN) and weight.shape == (D_OUT, D_IN)
    nc = _get_program((N_CORES, T_LOC, D_IN, D_OUT))
    x2d = np.ascontiguousarray(x.reshape(T_TOTAL, D_IN), dtype=np.float32)
    in_maps = make_in_maps(x2d, weight, N_CORES, T_LOC)
    try:
        res = run_bass_kernel_spmd(nc, in_maps, list(range(N_CORES)),
                                   trace=False)
    except Exception:
        res = run_bass_kernel_spmd(nc, in_maps, list(range(N_CORES)),
                                   trace=False)
    out = np.concatenate(
        [res.results[c]["out"].astype(np.float32) for c in range(N_CORES)],
        axis=0)
    return np.ascontiguousarray(out.reshape(B, S, D_OUT))


# revision 22
# speedup vs baseline: 1.0342x; 1.0121x over previous
"""BitLinear baseline (layernorm -> sign(W - mean(W)) GEMM -> *beta) on 8 TRN2 cores.

Sharding: data-parallel over tokens. Each core gets 1024 of the 8192 tokens
(x pre-transposed on host to [D_in, T_loc] bf16) and the full W^T in bf16,
both ROTATED along the CONTRACTION dim (d_in) by c*512 rows per core. Since a
contraction is order-invariant, no un-rotation of the output is needed; the
rotation only makes rows 0..511 (i-tiles 0..3) core c's private 1/8 stats
shard of W, loaded once as four contiguous [128, 4096] row tiles that are
retained in SBUF and re-sliced by the main GEMM.

Device-side math (per core):
  One AllReduce of [sum(W), sum|W|, count(W>=0)] over the row shards:
    mu    = sum/N
    beta  = (sum|w| - mu*(2*count - N))/N   (|w-mu| identity, error O(mu^2))
  Stats are split across engines so the AllReduce triggers early: sum on PE
  (ones-matmuls folded into one PSUM bank), |w| on ACT + DVE with accum_out,
  count on DVE + GpSimd. x load (4 big DMAs) + token-stat matmuls fill the
  AllReduce latency window; x is then mean-centered IN PLACE (x - mean_x,
  rank-1 broadcast matmul + DVE subs), which absorbs the colsum correction
  term of layernorm into the GEMM itself:
    out[s,o] = a[s] * ((x[s,:] - mean_x[s]) @ sign(W-mu)^T)[o]
  with a[s] = beta/sqrt(var[s]+eps). The epilogue is a single DVE
  tensor_scalar multiply by a[s] straight out of PSUM, written as bf16
  (host upcasts). a[s] reaches partition layout via rank-1 transpose
  matmuls instead of a DRAM round-trip.
  Matmuls run in bf16 (sign values exact in bf16), accumulation in fp32 PSUM.
"""

import numpy as np
from contextlib import ExitStack

from concourse import bass, bacc, tile, mybir
from concourse.bass_utils import run_bass_kernel_spmd

F32 = mybir.dt.float32
BF16 = mybir.dt.bfloat16
P = 128
LN_EPS = 1e-5

B, S, D_IN, D_OUT = 4, 2048, 4096, 4096
N_CORES = 8
T_TOTAL = B * S
T_LOC = T_TOTAL // N_CORES


def build_program(n_cores, t_loc, d_in, d_out, oc_width=512):
    n_it = d_in // P            # i tiles (contraction)
    n_st = t_loc // P           # s tiles (tokens)
    n_oc = d_out // oc_width    # output-feature chunks
    n_wr = (d_in // n_cores) // P   # row tiles in the per-core stats shard
    n_xp = 4                    # x load pieces
    it_per_xp = n_it // n_xp
    grp_w = 4                   # i tiles per grouped wf DMA / sign op
    n_wg = n_it // grp_w
    inv_w = 1.0 / float(d_in * d_out)
    inv_d = 1.0 / float(d_in)
    half = d_out // 2
    groups = [list(range(n_cores))]
    AX = mybir.AxisListType.X
    ADD = mybir.AluOpType.add
    AF = mybir.ActivationFunctionType

    nc = bacc.Bacc("TRN2", target_bir_lowering=False, debug=False,
                   num_devices=n_cores)
    xt = nc.dram_tensor("xt", [d_in, t_loc], BF16, kind="ExternalInput").ap()
    wt = nc.dram_tensor("wt", [d_in, d_out], BF16, kind="ExternalInput").ap()
    out = nc.dram_tensor("out", [t_loc, d_out], BF16,
                         kind="ExternalOutput").ap()

    with tile.TileContext(nc) as tc, ExitStack() as ctx:
        const = ctx.enter_context(tc.tile_pool(name="const", bufs=1))
        persist = ctx.enter_context(tc.tile_pool(name="persist", bufs=1))
        dram = ctx.enter_context(tc.tile_pool(name="dram", bufs=1, space="DRAM"))

        ones_col_bf = const.tile([P, 1], BF16, tag="ones_col_bf")
        nc.vector.memset(ones_col_bf[:], 1.0)
        ones_col_f = const.tile([P, 1], F32, tag="ones_col_f")
        nc.vector.memset(ones_col_f[:], 1.0)
        ones_row_f = const.tile([1, P], F32, tag="ones_row_f")
        nc.vector.memset(ones_row_f[:], 1.0)
        ones_row_bf = const.tile([1, P], BF16, tag="ones_row_bf")
        nc.vector.memset(ones_row_bf[:], 1.0)
        one_f = const.tile([1, 1], F32, tag="one_f")
        nc.vector.memset(one_f[:], 1.0)
        eps_col = const.tile([P, 1], F32, tag="eps_col")
        nc.vector.memset(eps_col[:], LN_EPS)
        zero_c = const.tile([P, 1], F32, tag="zero_c")
        nc.vector.memset(zero_c[:], 0.0)
        warm = const.tile([1, 1], F32, tag="warm")
        # pre-warm the ACT spline table set so the first |w| pass doesn't
        # pay the table-load latency mid-prologue
        nc.scalar.activation(warm[:], one_f[:], AF.Abs, bias=zero_c[0:1])

        neg_mu = persist.tile([P, 1], F32, tag="neg_mu")
        beta_col = persist.tile([P, 1], F32, tag="beta_col")
        a_col = persist.tile([P, n_st], F32, tag="a_col")
        mx_row = persist.tile([1, t_loc], BF16, tag="mx_row")
        mx_bc = persist.tile([P, t_loc], BF16, tag="mx_bc")

        xbf_pool = ctx.enter_context(tc.tile_pool(name="xbf", bufs=1))
        wrow_pool = ctx.enter_context(tc.tile_pool(name="wrow", bufs=1))
        wload = ctx.enter_context(tc.tile_pool(name="wload", bufs=3))
        wbin_pool = ctx.enter_context(
            tc.tile_pool(name="wbin", bufs=2 * n_wg))
        outsb = ctx.enter_context(tc.tile_pool(name="outsb", bufs=4))

        # ---- Phase 1: W stats from the contiguous row shard (i-tiles 0..3) --
        # sum(w) on PE (folded mod-512 into one PSUM bank); |w| on ACT
        # (tiles 0-2, halves) + DVE (tile 3); count(w>=0) on DVE (tiles 0-1)
        # + GpSimd (tiles 2-3, halves) -- all engines chew in parallel so
        # the AllReduce triggers early.
        wrows = []
        ps12_ctx = ExitStack()
        ps12 = ps12_ctx.enter_context(
            tc.tile_pool(name="ps12", bufs=1, space="PSUM"))
        ps3_ctx = ExitStack()
        ps3 = ps3_ctx.enter_context(
            tc.tile_pool(name="ps3", bufs=1, space="PSUM"))
        with tc.tile_pool(name="wstat", bufs=1) as wstat, \
             tc.tile_pool(name="wscr", bufs=1) as wscr:
            aacc = wstat.tile([P, 2 * n_wr], F32, tag="aacc")
            iacc = wstat.tile([P, n_wr], F32, tag="iacc")
            ps_sum = ps12.tile([1, oc_width], F32, tag="ps_sum")
            ascr = wscr.tile([P, half], BF16, tag="ascr")
            vscr = wscr.tile([P, d_out], BF16, tag="vscr")
            gscr = wscr.tile([P, half], BF16, tag="gscr")
            for i in range(n_wr):
                wr = wrow_pool.tile([P, d_out], BF16, tag=f"wrow{i}")
                nc.sync.dma_start(wr[:], wt[i * P:(i + 1) * P, :])
                wrows.append(wr)
                for j in range(n_oc):
                    nc.tensor.matmul(
                        ps_sum[:], ones_col_bf[:],
                        wr[:, j * oc_width:(j + 1) * oc_width],
                        start=(i == 0 and j == 0),
                        stop=(i == n_wr - 1 and j == n_oc - 1))
                for h in range(2):      # |w| on ACT, by halves
                    nc.scalar.activation(
                        ascr[:], wr[:, h * half:(h + 1) * half],
                        AF.Abs, bias=zero_c[:],
                        accum_out=aacc[:, 2 * i + h:2 * i + h + 1])
                nc.vector.tensor_scalar(vscr[:], wr[:], 0.0, 0.0,
                                        mybir.AluOpType.is_ge, ADD,
                                        accum_out=iacc[:, i:i + 1])
            s3 = wstat.tile([P, 2], F32, tag="s3")
            nc.vector.tensor_reduce(s3[:, 0:1], aacc[:], axis=AX, op=ADD)
            nc.vector.tensor_reduce(s3[:, 1:2], iacc[:], axis=AX, op=ADD)
            # one shared small-PSUM tile (bank-granular allocator): cols
            # 0..15 = transposed token sums, 16 = -mu bcast, 17 = beta
            # bcast, 18..19 = [sum|w|, count] totals on partition 0
            ps_tc = ps12.tile([P, 2 * n_st + 4], F32, tag="ps_tc")
            ps_tot = ps_tc[0:1, 2 * n_st + 2:2 * n_st + 4]
            nc.tensor.matmul(ps_tot, ones_col_f[:], s3[:])
            sb_tot = wstat.tile([1, 3], F32, tag="sb_tot")
            sscr = wstat.tile([1, oc_width], F32, tag="sscr")
            nc.scalar.activation(sscr[:], ps_sum[:], AF.Copy, bias=0.0,
                                 accum_out=sb_tot[:, 0:1])
            nc.vector.tensor_copy(sb_tot[:, 1:3], ps_tot)
            ar_in = dram.tile([1, 3], F32, tag="ar_in")
            ar_out = dram.tile([1, 3], F32, tag="ar_out")
            nc.scalar.dma_start(ar_in[:], sb_tot[:])
            nc.gpsimd.collective_compute(
                "AllReduce", ADD, replica_groups=groups,
                ins=[ar_in.opt()], outs=[ar_out.opt()])

        # ---- Phase 2: x load (bf16, 4 big DMAs) + token stats on PE --------
        xpieces = []
        n_ch = t_loc // 512
        statsb_ctx = ExitStack()
        statsb = statsb_ctx.enter_context(tc.tile_pool(name="statsb", bufs=1))
        with tc.tile_pool(name="x2p", bufs=2) as x2p:
            ps_s = ps3.tile([1, t_loc], F32, tag="ps_s")
            ps_s2 = ps3.tile([1, t_loc], F32, tag="ps_s2")
            for k in range(n_xp):
                xp = xbf_pool.tile([P, it_per_xp, t_loc], BF16, tag=f"xp{k}")
                nc.sync.dma_start(
                    xp[:],
                    xt[k * it_per_xp * P:(k + 1) * it_per_xp * P, :]
                    .rearrange("(j p) t -> p j t", p=P))
                xpieces.append(xp)
                for j in range(it_per_xp):
                    i = k * it_per_xp + j
                    x2 = x2p.tile([P, t_loc], BF16, tag="x2")
                    if i % 2 == 0:
                        nc.scalar.square(x2[:], xp[:, j, :])
                    else:
                        nc.vector.tensor_mul(x2[:], xp[:, j, :], xp[:, j, :])
                    for c in range(n_ch):
                        sl = slice(c * 512, (c + 1) * 512)
                        nc.tensor.matmul(ps_s[:, sl], ones_col_bf[:],
                                         xp[:, j, sl],
                                         start=(i == 0), stop=(i == n_it - 1))
                        nc.tensor.matmul(ps_s2[:, sl], ones_col_bf[:],
                                         x2[:, sl],
                                         start=(i == 0), stop=(i == n_it - 1))

            def xtile(i):
                return xpieces[i // it_per_xp][:, i % it_per_xp, :]

            # ---- token-stat rows -> SBUF; mean-center x in place -----------
            srow = statsb.tile([1, t_loc], F32, tag="srow")
            nc.vector.tensor_copy(srow[:], ps_s[:])
            s2row = statsb.tile([1, t_loc], F32, tag="s2row")
            nc.vector.tensor_copy(s2row[:], ps_s2[:])
            # mx = -mean_x, broadcast to all partitions via rank-1 matmul
            nc.scalar.activation(mx_row[:], srow[:], AF.Copy, scale=-inv_d)
            ps_mx = ps12.tile([P, t_loc], F32, tag="ps_mx")
            for c in range(n_ch):
                sl = slice(c * 512, (c + 1) * 512)
                nc.tensor.matmul(ps_mx[:, sl], ones_row_bf[:], mx_row[0:1, sl])
            nc.vector.tensor_copy(mx_bc[:], ps_mx[:])
            for i in range(n_it):
                nc.vector.tensor_add(xtile(i), xtile(i), mx_bc[:])

            # ---- transpose token sums to partition layout via rank-1 MMs ---
            for s in range(n_st):
                nc.tensor.matmul(ps_tc[:, s:s + 1],
                                 srow[0:1, s * P:(s + 1) * P], one_f[:])
                nc.tensor.matmul(ps_tc[:, n_st + s:n_st + s + 1],
                                 s2row[0:1, s * P:(s + 1) * P], one_f[:])

            # ---- Post-AR scalars (beta, -mu) -------------------------------
            tot = statsb.tile([1, 3], F32, tag="tot")
            nc.scalar.dma_start(tot[:], ar_out[:])
            ps_b = ps_tc[:, 2 * n_st:2 * n_st + 1]
            nc.tensor.matmul(ps_b, ones_row_f[:], tot[:, 0:1])
            nc.scalar.mul(neg_mu[:], ps_b, -inv_w)
            mu_sb = statsb.tile([1, 1], F32, tag="mu_sb")
            nc.scalar.mul(mu_sb[:], tot[:, 0:1], inv_w)
            sgn_t = statsb.tile([1, 1], F32, tag="sgn_t")
            nc.scalar.activation(sgn_t[:], tot[:, 2:3], AF.Copy,
                                 scale=2.0, bias=-float(d_in * d_out))
            t1 = statsb.tile([1, 1], F32, tag="t1")
            nc.vector.tensor_mul(t1[:], mu_sb[:], sgn_t[:])
            t2 = statsb.tile([1, 1], F32, tag="t2")
            nc.vector.tensor_sub(t2[:], tot[:, 1:2], t1[:])
            beta_sb = statsb.tile([1, 1], F32, tag="beta_sb")
            nc.scalar.mul(beta_sb[:], t2[:], inv_w)
            ps_bc = ps_tc[:, 2 * n_st + 1:2 * n_st + 2]
            nc.tensor.matmul(ps_bc, ones_row_f[:], beta_sb[:])
            nc.vector.tensor_copy(beta_col[:], ps_bc)

            # ---- a_col in partition layout ---------------------------------
            mu_c = statsb.tile([P, n_st], F32, tag="mu_c")
            nc.vector.tensor_scalar_mul(mu_c[:], ps_tc[:, 0:n_st], inv_d)
            e2_c = statsb.tile([P, n_st], F32, tag="e2_c")
            nc.vector.tensor_scalar_mul(e2_c[:], ps_tc[:, n_st:2 * n_st],
                                        inv_d)
            m2_c = statsb.tile([P, n_st], F32, tag="m2_c")
            nc.vector.tensor_mul(m2_c[:], mu_c[:], mu_c[:])
            var_c = statsb.tile([P, n_st], F32, tag="var_c")
            nc.vector.tensor_sub(var_c[:], e2_c[:], m2_c[:])
            sd_c = statsb.tile([P, n_st], F32, tag="sd_c")
            nc.scalar.activation(sd_c[:], var_c[:], AF.Sqrt, bias=eps_col[:])
            rs_c = statsb.tile([P, n_st], F32, tag="rs_c")
            nc.vector.reciprocal(rs_c[:], sd_c[:])
            nc.vector.tensor_scalar_mul(a_col[:], rs_c[:], beta_col[:])

        # ---- Phase 3: main GEMM over o-chunks ------------------------------
        statsb_ctx.close()
        ps3_ctx.close()
        ps12_ctx.close()
        ps_main = ctx.enter_context(
            tc.tile_pool(name="ps4", bufs=7, space="PSUM"))

        def emit_signs(oc):
            # one [128, grp_w*oc_width] sign tile per group of grp_w i-tiles
            o0 = oc * oc_width
            wgs = []
            for g in range(n_wg):
                wg = wbin_pool.tile([P, grp_w, oc_width], BF16, tag="wb",
                                    name="wb")
                if g == 0:
                    for j in range(grp_w):
                        nc.scalar.activation(wg[:, j, :],
                                             wrows[j][:, o0:o0 + oc_width],
                                             AF.Sign, bias=neg_mu[:])
                else:
                    wf = wload.tile([P, grp_w, oc_width], BF16, tag="wf")
                    nc.sync.dma_start(
                        wf[:],
                        wt[g * grp_w * P:(g + 1) * grp_w * P, o0:o0 + oc_width]
                        .rearrange("(j p) o -> p j o", p=P))
                    nc.scalar.activation(wg[:, :, :], wf[:], AF.Sign,
                                         bias=neg_mu[:])
                wgs.append(wg)
            return wgs

        def wbtile(wgs, i):
            return wgs[i // grp_w][:, i % grp_w, :]

        def emit_epilogue(po, s, o0):
            ob = outsb.tile([P, oc_width], BF16, tag="ob")
            nc.vector.tensor_scalar_mul(ob[:], po[:], a_col[:, s:s + 1])
            nc.gpsimd.dma_start(out[s * P:(s + 1) * P, o0:o0 + oc_width],
                                ob[:])

        for oc in range(n_oc):
            o0 = oc * oc_width
            wgs = emit_signs(oc)
            if oc == 0:
                # i-outer: consume sign tiles as they stream out of ACT
                grp = min(4, n_st)
                for h in range(0, n_st, grp):
                    pos = [ps_main.tile([P, oc_width], F32, tag="po",
                                        name="po") for _ in range(grp)]
                    for i in range(n_it):
                        for g in range(grp):
                            s = h + g
                            nc.tensor.matmul(
                                pos[g][:], xtile(i)[:, s * P:(s + 1) * P],
                                wbtile(wgs, i)[:],
                                start=(i == 0), stop=(i == n_it - 1))
                    for g in range(grp):
                        emit_epilogue(pos[g], h + g, o0)
            else:
                for s in range(n_st):
                    po = ps_main.tile([P, oc_width], F32, tag="po")
                    for i in range(n_it):
                        nc.tensor.matmul(po[:],
                                         xtile(i)[:, s * P:(s + 1) * P],
                                         wbtile(wgs, i)[:],
                                         start=(i == 0), stop=(i == n_it - 1))
                    emit_epilogue(po, s, o0)

    nc.compile()
    return nc


_PROGRAM_CACHE = {}


def _get_program(key):
    if key not in _PROGRAM_CACHE:
        _PROGRAM_CACHE[key] = build_program(*key)
    return _PROGRAM_CACHE[key]


def make_in_maps(x2d, weight, n_cores, t_loc):
    """Host-side sharding: token shards of x^T in bf16; full W^T in bf16.
    Both are rotated along d_in by c*(d_in/n_cores) rows so the program's
    row tiles 0..3 are core c's private 1/8 stats shard. Rotating the
    contraction dim leaves the output invariant, so no un-rotation."""
    bf16 = mybir.dt.np(BF16)
    d_in = x2d.shape[1]
    roll = d_in // n_cores
    wt_full = np.ascontiguousarray(weight.T).astype(bf16)
    in_maps = []
    for c in range(n_cores):
        xt_c = np.ascontiguousarray(
            np.roll(x2d[c * t_loc:(c + 1) * t_loc, :].T, -c * roll, axis=0)
        ).astype(bf16)
        wt_c = np.ascontiguousarray(np.roll(wt_full, -c * roll, axis=0))
        in_maps.append({"xt": xt_c, "wt": wt_c})
    return in_maps


def kernel(x: np.ndarray, weight: np.ndarray) -> np.ndarray:
    assert x.shape == (B, S, D_IN) and weight.shape == (D_OUT, D_IN)
    nc = _get_program((N_CORES, T_LOC, D_IN, D_OUT))
    x2d = np.ascontiguousarray(x.reshape(T_TOTAL, D_IN), dtype=np.float32)
    in_maps = make_in_maps(x2d, weight, N_CORES, T_LOC)
    try:
        res = run_bass_kernel_spmd(nc, in_maps, list(range(N_CORES)),
                                   trace=False)
    except Exception:
        res = run_bass_kernel_spmd(nc, in_maps, list(range(N_CORES)),
                                   trace=False)
    out = np.concatenate(
        [res.results[c]["out"].astype(np.float32) for c in range(N_CORES)],
        axis=0)
    return np.ascontiguousarray(out.reshape(B, S, D_OUT))
